# revision 17
# baseline (speedup 1.0000x reference)
"""Trainium2 Bass kernel for a single-head transformer encoder layer.

Problem shapes (hardcoded): B=4, S=4096, D=512, D_FFN=2048, fp32.
Sharding: 8 cores; core c handles batch b=c//2, query-row half h=c%2
(2048 q rows each). K/V for the batch's full sequence are handled
on-core (duplicated across the 2 cores sharing a batch).

v5 structure (fp8 attention via DoubleRow, bf16 FFN, host algebra):
  host: G = Wq@Wk^T (applied transposed on device) folds both score
        projections into one; exp bias handles bq exactly (bk drops
        out of softmax); q/k uploaded pre-transposed bf16; v/W* bf16.
  DMA: spread across the three dispatch queues (SP: G/qT/v, ACT: kT,
       Pool: weights/x) so the score path isn't serialized.
  pass 1: kT -> fp8 (ACT casts); qT -> fp8 -> project by G (fp8
          DoubleRow) -> qgt fp8; v natural -> fp8. No PE transposes.
  pass 2: per 512-q block: scoresT[k,q] via fp8 DoubleRow; exp on ACT
          (shift C cancels in normalization) -> ptile fp8; Z^T = v^T P
          fp8 DoubleRow accumulated over 32 k-chunks in 4 PSUM banks;
          row sums via ones DoubleRow matmul in a 5th bank.
          attn = (Z@Wv) fp8 DoubleRow, scaled 1/rsum on drain, +x into
          u rows; LN1 stats inline, single sqrt at pass end, apply
          deferred into pass 3 (no ACT table thrash against exp).
  pass 3: per 512-row block: LN1-apply -> h bf16, PE-transpose h,
          FFN1 (relu+bias in ACT) bf16, FFN2 bf16, +h residual with
          inline LN2 stats, batched sqrt, store per row-chunk.
  Ops for zero biases / unit gamma / zero beta are skipped when the
  runtime values allow (general path kept otherwise).
"""

import math
import threading
from contextlib import ExitStack

import ml_dtypes
import numpy as np

import concourse.bass as bass
import concourse.tile as tile
from concourse import bacc, mybir
from concourse.bass_utils import run_bass_kernel_spmd
from concourse.masks import make_identity

P = 128
B, S, D = 4, 4096, 512
F = 4 * D                    # 2048
M = S // 2                   # q rows per core
DC = D // P                  # 4 feature chunks
FC = F // P                  # 16 ffn chunks
SC = S // P                  # 32 k chunks
RC = M // P                  # 16 row chunks per core
QB = 512                     # q-block cols
NQB = M // QB                # 4
EPS = 1e-5
SCALE = 1.0 / math.sqrt(D)
CSHIFT = 2.5                 # exp shift; cancels in softmax normalization
f32 = mybir.dt.float32
bf16 = mybir.dt.bfloat16
fp8 = mybir.dt.float8e4
N_CORES = 8
DR = mybir.MatmulPerfMode.DoubleRow
Exp = mybir.ActivationFunctionType.Exp
Copy = mybir.ActivationFunctionType.Copy
Relu = mybir.ActivationFunctionType.Relu
Sqrt = mybir.ActivationFunctionType.Sqrt


def _bcast_load(nc, pool, vec_ap, n, tag):
    t = pool.tile([P, n], f32, tag=tag)
    src = bass.AP(tensor=vec_ap.tensor, offset=vec_ap.offset,
                  ap=[[0, P]] + list(vec_ap.ap))
    nc.gpsimd.dma_start(out=t[:], in_=src)
    return t


def build_program(spec):
    """spec: frozenset of flags among {'bv0','b20','bq0','ln1_triv',
    'ln2_triv'} marking inputs that are exactly zero / identity."""
    nc = bacc.Bacc()
    # All tensors are pre-shuffled on the host into the exact SBUF
    # layout ([128, free] contiguous per partition) so every load is one
    # maximally-aggregated DMA. fp8 tensors travel as uint8 (PJRT path
    # rejects fp8 params) and are bitcast to fp8 at the DMA source APs.
    u32 = mybir.dt.uint32
    qT = nc.dram_tensor("qT", [P, DC * M // 4], u32, kind="ExternalInput")
    kT = nc.dram_tensor("kT", [P, DC * S // 4], u32, kind="ExternalInput")
    v = nc.dram_tensor("v", [P, SC * D // 4], u32, kind="ExternalInput")
    x = nc.dram_tensor("x", [P, RC * D], f32, kind="ExternalInput")
    G = nc.dram_tensor("G", [P, DC * D // 4], u32, kind="ExternalInput")
    Wv = nc.dram_tensor("Wv", [P, DC * D // 4], u32, kind="ExternalInput")
    W1 = nc.dram_tensor("W1", [P, DC * F], bf16, kind="ExternalInput")
    W2 = nc.dram_tensor("W2", [P, FC * D], bf16, kind="ExternalInput")
    b1 = nc.dram_tensor("b1", [P, FC], f32, kind="ExternalInput")
    abias = (None if "bq0" in spec else
             nc.dram_tensor("abias", [P, SC], f32, kind="ExternalInput"))
    bv = (None if "bv0" in spec else
          nc.dram_tensor("bv", [D], f32, kind="ExternalInput"))
    b2 = (None if "b20" in spec else
          nc.dram_tensor("b2", [D], f32, kind="ExternalInput"))
    if "ln1_triv" not in spec:
        g1 = nc.dram_tensor("gamma1", [D], f32, kind="ExternalInput")
        be1 = nc.dram_tensor("beta1", [D], f32, kind="ExternalInput")
    if "ln2_triv" not in spec:
        g2 = nc.dram_tensor("gamma2", [D], f32, kind="ExternalInput")
        be2 = nc.dram_tensor("beta2", [D], f32, kind="ExternalInput")
    out = nc.dram_tensor("out", [M, D], f32, kind="ExternalOutput")

    with tile.TileContext(nc) as tc, ExitStack() as ctx:
        g_pool = ctx.enter_context(tc.tile_pool(name="glob", bufs=1))
        htp = ctx.enter_context(tc.tile_pool(name="htp", bufs=2))
        wp = ctx.enter_context(tc.tile_pool(name="wp", bufs=1))
        ep = ctx.enter_context(tc.tile_pool(name="ep", bufs=2))

        ident_t = g_pool.tile([P, P], bf16, tag="ident")
        make_identity(nc, ident_t[:])
        ident = ident_t[:]
        ones8 = g_pool.tile([P, 2, 4], fp8, tag="ones8")
        nc.vector.memset(ones8[:], 1.0)
        eps_t = g_pool.tile([P, 1], f32, tag="eps")
        nc.vector.memset(eps_t[:], EPS)
        negc_t = g_pool.tile([P, 1], f32, tag="negc")
        nc.vector.memset(negc_t[:], -CSHIFT)
        h_bf = g_pool.tile([P, RC, D], bf16, tag="h_bf")
        mv1 = g_pool.tile([P, RC, 2], f32, tag="mv1")

        # Queue SP (nc.sync): G, qT, v -- the early PE critical path.
        # Queue ACT (nc.scalar): kT. Queue Pool (nc.gpsimd): x, weights.
        g8 = wp.tile([P, DC, D], fp8, tag="g8")
        nc.gpsimd.dma_start(g8[:].bitcast(mybir.dt.uint32), G[:, :])
        x_full = g_pool.tile([P, RC, D], f32, tag="x_full")
        wv8 = wp.tile([P, DC, D], fp8, tag="wv8")
        nc.sync.dma_start(wv8[:].bitcast(mybir.dt.uint32), Wv[:, :])
        w1_sb = wp.tile([P, DC, F], bf16, tag="w1")
        nc.sync.dma_start(w1_sb[:], W1[:, :])
        w2_sb = wp.tile([P, FC, D], bf16, tag="w2")
        nc.sync.dma_start(w2_sb[:], W2[:, :])
        b1_fm = wp.tile([P, FC], f32, tag="b1")
        nc.sync.dma_start(b1_fm[:], b1[:, :])
        if abias is not None:
            abias_fm = wp.tile([P, SC], f32, tag="abias")
            nc.sync.dma_start(abias_fm[:], abias[:, :])
        else:
            abias_fm = None

        with ExitStack() as actx:
            attn = actx.enter_context(tc.tile_pool(name="attn", bufs=1))
            kt8 = attn.tile([P, DC, S], fp8, tag="kt8")
            qgt = attn.tile([P, DC, M], fp8, tag="qgt")
            v8 = attn.tile([P, SC, D], fp8, tag="v8")

            # ---------- pass 1: direct fp8 loads + q-side G projection ----
            with ExitStack() as p1ctx:
                qt8_full = attn.tile([P, DC, M], fp8, tag="qt8f")
                u32 = mybir.dt.uint32
                nc.gpsimd.dma_start(qt8_full[:].bitcast(u32), qT[:, :])
                kr = kT[:, :].rearrange("p (c r) -> p c r", r=S // 4)
                k32 = kt8[:].bitcast(u32)
                nc.scalar.dma_start(k32[:, 0:2, :], kr[:, 0:2, :])
                nc.gpsimd.dma_start(k32[:, 2:4, :], kr[:, 2:4, :])
                vr = v[:, :].rearrange("p (c d) -> p c d", d=D // 4)
                v32 = v8[:].bitcast(u32)
                nc.gpsimd.dma_start(v32[:, 0:16, :], vr[:, 0:16, :])
                nc.gpsimd.dma_start(v32[:, 16:32, :], vr[:, 16:32, :])
                nc.gpsimd.dma_start(x_full[:], x[:, :])
                ps_p = p1ctx.enter_context(
                    tc.tile_pool(name="ps_p", bufs=3, space="PSUM"))
                for j in range(NQB):
                    for m in range(DC):
                        psp = ps_p.tile([P, QB], f32, tag="ps_proj")
                        for c2 in (0, 2):
                            nc.tensor.matmul(
                                psp[:],
                                lhsT=g8[:, c2:c2 + 2, m * P:(m + 1) * P],
                                rhs=qt8_full[:, c2:c2 + 2,
                                             j * QB:(j + 1) * QB],
                                start=(c2 == 0), stop=(c2 == 2),
                                perf_mode=DR)
                        nc.vector.tensor_copy(
                            qgt[:, m, j * QB:(j + 1) * QB], psp[:])

            # ---------- pass 2: attention; LN1 stats inline ----------
            with ExitStack() as p2ctx:
                p2 = p2ctx.enter_context(tc.tile_pool(name="ph2", bufs=1))
                ptp = p2ctx.enter_context(tc.tile_pool(name="ptp", bufs=3))
                zsp = p2ctx.enter_context(tc.tile_pool(name="zsp", bufs=2))
                ps_s = p2ctx.enter_context(
                    tc.tile_pool(name="ps_s", bufs=3, space="PSUM"))
                ps_z = p2ctx.enter_context(
                    tc.tile_pool(name="ps_z", bufs=1, space="PSUM"))
                ps_r = p2ctx.enter_context(
                    tc.tile_pool(name="ps_r", bufs=1, space="PSUM"))

                bv_bc = (None if bv is None else
                         _bcast_load(nc, p2, bv[:], D, "bv"))
                if "ln1_triv" not in spec:
                    g1a_bc = _bcast_load(nc, p2, g1[:], D, "g1a")
                    be1a_bc = _bcast_load(nc, p2, be1[:], D, "be1a")

                def _ln1_finish(lo, hi):
                    nc.scalar.activation(
                        out=mv1[:, lo:hi, 1:2], in_=mv1[:, lo:hi, 1:2],
                        func=Sqrt, bias=eps_t[:], scale=1.0, alpha=0.0)
                    nc.vector.reciprocal(out=mv1[:, lo:hi, 1],
                                         in_=mv1[:, lo:hi, 1])

                def _h_apply(rc):
                    hrow = h_bf[:, rc, :]
                    nc.vector.tensor_scalar(
                        out=hrow, in0=x_full[:, rc, :],
                        scalar1=mv1[:, rc, 0:1], scalar2=mv1[:, rc, 1:2],
                        op0=mybir.AluOpType.subtract,
                        op1=mybir.AluOpType.mult)
                    if "ln1_triv" not in spec:
                        nc.vector.tensor_mul(out=hrow, in0=hrow,
                                             in1=g1a_bc[:])
                        nc.vector.tensor_add(out=hrow, in0=hrow,
                                             in1=be1a_bc[:])

                for qb in range(NQB):
                    if qb == NQB - 1:
                        _ln1_finish(0, 12)
                    psZ = ps_z.tile([P, DC, QB], f32, tag="psZ")
                    pr = ps_r.tile([P, 4, 4], f32, tag="pr")
                    NP2 = SC // 2

                    def _zt_pair(kp, pt):
                        for dc in range(DC):
                            nc.tensor.matmul(
                                psZ[:, dc, :],
                                lhsT=v8[:, 2 * kp:2 * kp + 2,
                                        dc * P:(dc + 1) * P],
                                rhs=pt[:],
                                start=(kp == 0), stop=(kp == NP2 - 1),
                                perf_mode=DR)
                        for qc in range(4):
                            nc.tensor.matmul(
                                pr[:, qc, :],
                                lhsT=pt[:, :, qc * P:(qc + 1) * P],
                                rhs=ones8[:],
                                start=(kp == 0), stop=(kp == NP2 - 1),
                                perf_mode=DR)

                    pt_prev = None
                    for kp in range(NP2):
                        ptile = ptp.tile([P, 2, QB], fp8, tag="pt")
                        for sub in range(2):
                            kc = 2 * kp + sub
                            pss = ps_s.tile([P, QB], f32, tag="pss")
                            for d2 in (0, 2):
                                nc.tensor.matmul(
                                    pss[:],
                                    lhsT=kt8[:, d2:d2 + 2,
                                             kc * P:(kc + 1) * P],
                                    rhs=qgt[:, d2:d2 + 2,
                                            qb * QB:(qb + 1) * QB],
                                    start=(d2 == 0), stop=(d2 == 2),
                                    perf_mode=DR)
                            ebias = (negc_t[:] if abias_fm is None
                                     else abias_fm[:, kc:kc + 1])
                            nc.scalar.activation(
                                out=ptile[:, sub, :], in_=pss[:],
                                func=Exp, bias=ebias, scale=SCALE, alpha=0.0)
                        if pt_prev is not None:
                            _zt_pair(kp - 1, pt_prev)
                        pt_prev = ptile
                    _zt_pair(NP2 - 1, pt_prev)
                    rsum_sb = ep.tile([P, 4], f32, tag="rsum_sb")
                    nc.vector.tensor_copy(rsum_sb[:], pr[:, :, 0])
                    rinv = ep.tile([P, 4], f32, tag="rinv")
                    nc.vector.reciprocal(out=rinv[:], in_=rsum_sb[:])
                    z8 = zsp.tile([P, DC, QB], fp8, tag="z8")
                    psAs = []
                    for qc in range(4):
                        nc.vector.tensor_copy(
                            z8[:, :, qc * P:(qc + 1) * P],
                            psZ[:, :, qc * P:(qc + 1) * P])
                        psA = ps_s.tile([P, QB], f32, tag="pss", name="psA")
                        for c2 in (0, 2):
                            nc.tensor.matmul(
                                psA[:],
                                lhsT=z8[:, c2:c2 + 2, qc * P:(qc + 1) * P],
                                rhs=wv8[:, c2:c2 + 2, :],
                                start=(c2 == 0), stop=(c2 == 2),
                                perf_mode=DR)
                        psAs.append(psA)
                    if qb == NQB - 1:
                        for rc0 in range(4):
                            _h_apply(rc0)
                    for qc in range(4):
                        rc = qb * 4 + qc
                        t = x_full[:, rc, :]
                        nc.vector.scalar_tensor_tensor(
                            out=t, in0=psAs[qc][:],
                            scalar=rinv[:, qc:qc + 1],
                            in1=t, op0=mybir.AluOpType.mult,
                            op1=mybir.AluOpType.add)
                        if bv_bc is not None:
                            nc.vector.tensor_add(out=t, in0=t, in1=bv_bc[:])
                        stats = ep.tile([P, 6], f32, tag="ln_stats")
                        nc.vector.bn_stats(out=stats[:], in_=t)
                        nc.vector.bn_aggr(out=mv1[:, rc, :], in_=stats[:])

                _ln1_finish(12, RC)

        # ---------- pass 3: LN1-apply, FFN, LN2, store ----------
        with ExitStack() as p3ctx:
            p3 = p3ctx.enter_context(tc.tile_pool(name="ph3", bufs=1))
            f1p = p3ctx.enter_context(tc.tile_pool(name="f1p", bufs=1))
            ps_f = p3ctx.enter_context(
                tc.tile_pool(name="ps_f", bufs=2, space="PSUM"))
            ps_g = p3ctx.enter_context(
                tc.tile_pool(name="ps_g", bufs=3, space="PSUM"))
            ps_t = p3ctx.enter_context(
                tc.tile_pool(name="ps_t", bufs=2, space="PSUM"))

            if "ln1_triv" in spec:
                g1_bc = be1_bc = None
            else:
                g1_bc = _bcast_load(nc, p3, g1[:], D, "g1")
                be1_bc = _bcast_load(nc, p3, be1[:], D, "be1")
            if "ln2_triv" in spec:
                g2_bc = be2_bc = None
            else:
                g2_bc = _bcast_load(nc, p3, g2[:], D, "g2")
                be2_bc = _bcast_load(nc, p3, be2[:], D, "be2")
            b2_bc = (None if b2 is None else
                     _bcast_load(nc, p3, b2[:], D, "b2"))

            def _prep_block(fb):
                """LN1-apply each row into h_bf, PE-transpose it."""
                htr = htp.tile([P, DC, QB], bf16, tag="ht_blk",
                               name=f"htl{fb}")
                for qc in range(4):
                    rc = fb * 4 + qc
                    hrow = h_bf[:, rc, :]
                    if fb > 0:
                        nc.vector.tensor_scalar(
                            out=hrow, in0=x_full[:, rc, :],
                            scalar1=mv1[:, rc, 0:1], scalar2=mv1[:, rc, 1:2],
                            op0=mybir.AluOpType.subtract,
                            op1=mybir.AluOpType.mult)
                        if g1_bc is not None:
                            nc.vector.tensor_mul(out=hrow, in0=hrow,
                                                 in1=g1_bc[:])
                            nc.vector.tensor_add(out=hrow, in0=hrow,
                                                 in1=be1_bc[:])
                    pst = ps_t.tile([P, DC, P], bf16, tag="ps_tp")
                    for dc in range(DC):
                        nc.tensor.transpose(
                            pst[:, dc, :],
                            h_bf[:, rc, dc * P:(dc + 1) * P], ident)
                    nc.vector.tensor_copy(htr[:, :, qc * P:(qc + 1) * P],
                                          pst[:])
                return htr

            mv2 = p3.tile([P, RC, 2], f32, tag="mv2")
            ht_next = _prep_block(0)
            for fb in range(NQB):
                htr = ht_next
                f1t = f1p.tile([P, FC, QB], bf16, tag="f1t")
                for fc in range(FC):
                    psf = ps_f.tile([P, QB], f32, tag="ps_ffn")
                    for dc in range(DC):
                        nc.tensor.matmul(
                            psf[:], lhsT=w1_sb[:, dc, fc * P:(fc + 1) * P],
                            rhs=htr[:, dc, :],
                            start=(dc == 0), stop=(dc == DC - 1))
                    nc.scalar.activation(
                        out=f1t[:, fc, :], in_=psf[:],
                        func=Relu, bias=b1_fm[:, fc:fc + 1], scale=1.0,
                        alpha=0.0)
                if fb + 1 < NQB:
                    ht_next = _prep_block(fb + 1)
                u2 = p3.tile([P, 4, D], f32, tag="u2", name=f"u2_{fb}")
                last = fb == NQB - 1

                def _ln2_emit(qcs):
                    fs0 = fb * 4 + qcs[0]
                    fs1 = fb * 4 + qcs[-1] + 1
                    nc.scalar.activation(
                        out=mv2[:, fs0:fs1, 1:2], in_=mv2[:, fs0:fs1, 1:2],
                        func=Sqrt, bias=eps_t[:], scale=1.0, alpha=0.0)
                    nc.vector.reciprocal(out=mv2[:, fs0:fs1, 1],
                                         in_=mv2[:, fs0:fs1, 1])
                    for qc in qcs:
                        rc = fb * 4 + qc
                        res_t = ep.tile([P, D], f32, tag="res_t")
                        nc.vector.tensor_scalar(
                            out=res_t[:], in0=u2[:, qc, :],
                            scalar1=mv2[:, rc, 0:1], scalar2=mv2[:, rc, 1:2],
                            op0=mybir.AluOpType.subtract,
                            op1=mybir.AluOpType.mult)
                        if g2_bc is not None:
                            nc.vector.tensor_mul(out=res_t[:], in0=res_t[:],
                                                 in1=g2_bc[:])
                            nc.vector.tensor_add(out=res_t[:], in0=res_t[:],
                                                 in1=be2_bc[:])
                        eng = (nc.sync, nc.gpsimd, nc.scalar)[qc % 3]
                        eng.dma_start(out[rc * P:(rc + 1) * P, :], res_t[:])

                for qc in range(4):
                    rc = fb * 4 + qc
                    pso = ps_g.tile([P, D], f32, tag="ps_out2")
                    for fc in range(FC):
                        nc.tensor.matmul(
                            pso[:], lhsT=f1t[:, fc, qc * P:(qc + 1) * P],
                            rhs=w2_sb[:, fc, :],
                            start=(fc == 0), stop=(fc == FC - 1))
                    if b2_bc is not None:
                        nc.vector.tensor_add(out=pso[:], in0=pso[:],
                                             in1=b2_bc[:])
                    nc.vector.tensor_add(out=u2[:, qc, :], in0=pso[:],
                                         in1=h_bf[:, rc, :])
                    stats = ep.tile([P, 6], f32, tag="ln_stats")
                    nc.vector.bn_stats(out=stats[:], in_=u2[:, qc, :])
                    nc.vector.bn_aggr(out=mv2[:, rc, :], in_=stats[:])
                    if last and qc >= 2:
                        # drain rows eagerly at the end of the kernel
                        _ln2_emit(list(range(2)) if qc == 2 else [2, 3])
                if not last:
                    _ln2_emit([0, 1, 2, 3])

    nc.finalize()
    return nc


_CACHE = {}
_LOCK = threading.Lock()


def _get_program(spec):
    with _LOCK:
        if spec not in _CACHE:
            _CACHE[spec] = build_program(spec)
        return _CACHE[spec]


def _spec_flags(inputs):
    flags = set()
    if not np.any(inputs["bq"]):
        flags.add("bq0")
    if not np.any(inputs["bv"]):
        flags.add("bv0")
    if not np.any(inputs["b2"]):
        flags.add("b20")
    if (np.all(inputs["gamma1"] == 1.0) and not np.any(inputs["beta1"])):
        flags.add("ln1_triv")
    if (np.all(inputs["gamma2"] == 1.0) and not np.any(inputs["beta2"])):
        flags.add("ln2_triv")
    return frozenset(flags)


def _shuf(a, chunks):
    """[chunks*128, n] row-major -> [128, chunks*n] partition-major."""
    n = a.shape[1]
    return np.ascontiguousarray(
        a.reshape(chunks, P, n).transpose(1, 0, 2).reshape(P, chunks * n))


def make_in_maps(inputs):
    bf = ml_dtypes.bfloat16
    f8 = ml_dtypes.float8_e4m3
    spec = _spec_flags(inputs)
    # device projection applies G_dev^T to q^T, so upload Wq@Wk^T to get
    # scoresT = k (Wk Wq^T) q^T = K Q^T
    G = (inputs["Wq"].astype(np.float32)
         @ inputs["Wk"].astype(np.float32).T).astype(f8)
    weights = {
        "G": _shuf(G, DC).view(np.uint32),
        "Wv": _shuf(inputs["Wv"].astype(f8), DC).view(np.uint32),
        "W1": _shuf(inputs["W1"].astype(bf), DC),
        "W2": _shuf(inputs["W2"].astype(bf), FC),
        "b1": np.ascontiguousarray(
            inputs["b1"].astype(np.float32).reshape(FC, P).T),
    }
    if "bv0" not in spec:
        weights["bv"] = np.ascontiguousarray(inputs["bv"].astype(np.float32))
    if "b20" not in spec:
        weights["b2"] = np.ascontiguousarray(inputs["b2"].astype(np.float32))
    if "ln1_triv" not in spec:
        weights["gamma1"] = np.ascontiguousarray(
            inputs["gamma1"].astype(np.float32))
        weights["beta1"] = np.ascontiguousarray(
            inputs["beta1"].astype(np.float32))
    if "ln2_triv" not in spec:
        weights["gamma2"] = np.ascontiguousarray(
            inputs["gamma2"].astype(np.float32))
        weights["beta2"] = np.ascontiguousarray(
            inputs["beta2"].astype(np.float32))
    wkbq = (None if "bq0" in spec else
            inputs["Wk"].astype(np.float32)
            @ inputs["bq"].astype(np.float32))
    in_maps = []
    for c in range(N_CORES):
        b, h = c // 2, c % 2
        sl = slice(h * M, (h + 1) * M)
        kb = inputs["k"][b].astype(np.float32)
        m = {
            "qT": _shuf(np.ascontiguousarray(inputs["q"][b, sl].T)
                        .astype(bf).astype(f8), DC).view(np.uint32),
            "kT": _shuf(np.ascontiguousarray(kb.T)
                        .astype(bf).astype(f8), DC).view(np.uint32),
            "v": _shuf(inputs["v"][b].astype(bf).astype(f8),
                       SC).view(np.uint32),
            "x": _shuf(inputs["x"][b, sl].astype(np.float32), RC),
            **weights,
        }
        if wkbq is not None:
            m["abias"] = np.ascontiguousarray(
                ((kb @ wkbq) * SCALE - CSHIFT)
                .astype(np.float32).reshape(SC, P).T)
        in_maps.append(m)
    return in_maps


def kernel(**inputs):
    spec = _spec_flags(inputs)
    nc = _get_program(spec)
    in_maps = make_in_maps(inputs)
    res = run_bass_kernel_spmd(nc, in_maps, list(range(N_CORES)))
    out = np.empty((B, S, D), np.float32)
    for c in range(N_CORES):
        b, h = c // 2, c % 2
        out[b, h * M:(h + 1) * M] = res.results[c]["out"]
    return out


# revision 18
# speedup vs baseline: 1.1767x; 1.1767x over previous
"""Trainium2 Bass kernel for a single-head transformer encoder layer.

Problem shapes (hardcoded): B=4, S=4096, D=512, D_FFN=2048, fp32.
Sharding: 8 cores; core c handles batch b=c//2, query-row half h=c%2
(2048 q rows each). K/V for the batch's full sequence are handled
on-core (duplicated across the 2 cores sharing a batch).

v5 structure (fp8 attention via DoubleRow, bf16 FFN, host algebra):
  host: G = Wq@Wk^T (applied transposed on device) folds both score
        projections into one; exp bias handles bq exactly (bk drops
        out of softmax); q/k uploaded pre-transposed bf16; v/W* bf16.
  DMA: spread across the three dispatch queues (SP: G/qT/v, ACT: kT,
       Pool: weights/x) so the score path isn't serialized.
  pass 1: kT -> fp8 (ACT casts); qT -> fp8 -> project by G (fp8
          DoubleRow) -> qgt fp8; v natural -> fp8. No PE transposes.
  pass 2: per 512-q block: scoresT[k,q] via fp8 DoubleRow; exp on ACT
          (shift C cancels in normalization) -> ptile fp8; Z^T = v^T P
          fp8 DoubleRow accumulated over 32 k-chunks in 4 PSUM banks;
          row sums via ones DoubleRow matmul in a 5th bank.
          attn = (Z@Wv) fp8 DoubleRow, scaled 1/rsum on drain, +x into
          u rows; LN1 stats inline, single sqrt at pass end, apply
          deferred into pass 3 (no ACT table thrash against exp).
  pass 3: per 512-row block: LN1-apply -> h bf16, PE-transpose h,
          FFN1 (relu+bias in ACT) bf16, FFN2 bf16, +h residual with
          inline LN2 stats, batched sqrt, store per row-chunk.
  Ops for zero biases / unit gamma / zero beta are skipped when the
  runtime values allow (general path kept otherwise).
"""

import math
import threading
from contextlib import ExitStack

import ml_dtypes
import numpy as np

import concourse.bass as bass
import concourse.tile as tile
from concourse import bacc, mybir
from concourse.bass_utils import run_bass_kernel_spmd
from concourse.masks import make_identity

P = 128
B, S, D = 4, 4096, 512
F = 4 * D                    # 2048
M = S // 2                   # q rows per core
DC = D // P                  # 4 feature chunks
FC = F // P                  # 16 ffn chunks
SC = S // P                  # 32 k chunks
RC = M // P                  # 16 row chunks per core
QB = 512                     # q-block cols
NQB = M // QB                # 4
EPS = 1e-5
SCALE = 1.0 / math.sqrt(D)
CSHIFT = 2.5                 # exp shift; cancels in softmax normalization
f32 = mybir.dt.float32
bf16 = mybir.dt.bfloat16
fp8 = mybir.dt.float8e4
N_CORES = 8
DR = mybir.MatmulPerfMode.DoubleRow
Exp = mybir.ActivationFunctionType.Exp
Copy = mybir.ActivationFunctionType.Copy
Relu = mybir.ActivationFunctionType.Relu
Sqrt = mybir.ActivationFunctionType.Sqrt


def _bcast_load(nc, pool, vec_ap, n, tag):
    t = pool.tile([P, n], f32, tag=tag)
    src = bass.AP(tensor=vec_ap.tensor, offset=vec_ap.offset,
                  ap=[[0, P]] + list(vec_ap.ap))
    nc.gpsimd.dma_start(out=t[:], in_=src)
    return t


def build_program(spec):
    """spec: frozenset of flags among {'bv0','b20','bq0','ln1_triv',
    'ln2_triv'} marking inputs that are exactly zero / identity."""
    nc = bacc.Bacc()
    # All tensors are pre-shuffled on the host into the exact SBUF
    # layout ([128, free] contiguous per partition) so every load is one
    # maximally-aggregated DMA. fp8 tensors travel as uint8 (PJRT path
    # rejects fp8 params) and are bitcast to fp8 at the DMA source APs.
    u32 = mybir.dt.uint32
    qT = nc.dram_tensor("qT", [P, DC * M // 4], u32, kind="ExternalInput")
    kT = nc.dram_tensor("kT", [P, DC * S // 4], u32, kind="ExternalInput")
    v = nc.dram_tensor("v", [P, SC * D // 4], u32, kind="ExternalInput")
    x = nc.dram_tensor("x", [P, RC * D], f32, kind="ExternalInput")
    G = nc.dram_tensor("G", [P, DC * D // 4], u32, kind="ExternalInput")
    Wv = nc.dram_tensor("Wv", [P, DC * D // 4], u32, kind="ExternalInput")
    W1 = nc.dram_tensor("W1", [P, DC * F], bf16, kind="ExternalInput")
    W2 = nc.dram_tensor("W2", [P, FC * D], bf16, kind="ExternalInput")
    b1 = nc.dram_tensor("b1", [P, FC], f32, kind="ExternalInput")
    abias = (None if "bq0" in spec else
             nc.dram_tensor("abias", [P, SC], f32, kind="ExternalInput"))
    bv = (None if "bv0" in spec else
          nc.dram_tensor("bv", [D], f32, kind="ExternalInput"))
    b2 = (None if "b20" in spec else
          nc.dram_tensor("b2", [D], f32, kind="ExternalInput"))
    if "ln1_triv" not in spec:
        g1 = nc.dram_tensor("gamma1", [D], f32, kind="ExternalInput")
        be1 = nc.dram_tensor("beta1", [D], f32, kind="ExternalInput")
    if "ln2_triv" not in spec:
        g2 = nc.dram_tensor("gamma2", [D], f32, kind="ExternalInput")
        be2 = nc.dram_tensor("beta2", [D], f32, kind="ExternalInput")
    out = nc.dram_tensor("out", [M, D], f32, kind="ExternalOutput")

    with tile.TileContext(nc) as tc, ExitStack() as ctx:
        g_pool = ctx.enter_context(tc.tile_pool(name="glob", bufs=1))
        htp = ctx.enter_context(tc.tile_pool(name="htp", bufs=2))
        wp = ctx.enter_context(tc.tile_pool(name="wp", bufs=1))
        ep = ctx.enter_context(tc.tile_pool(name="ep", bufs=2))

        ident_t = g_pool.tile([P, P], bf16, tag="ident")
        make_identity(nc, ident_t[:])
        ident = ident_t[:]
        ones8 = g_pool.tile([P, 2, 4], fp8, tag="ones8")
        nc.vector.memset(ones8[:], 1.0)
        eps_t = g_pool.tile([P, 1], f32, tag="eps")
        nc.vector.memset(eps_t[:], EPS)
        negc_t = g_pool.tile([P, 1], f32, tag="negc")
        nc.vector.memset(negc_t[:], -CSHIFT)
        h_bf = g_pool.tile([P, RC, D], bf16, tag="h_bf")
        mv1 = g_pool.tile([P, RC, 2], f32, tag="mv1")

        # Queue SP (nc.sync): G, qT, v -- the early PE critical path.
        # Queue ACT (nc.scalar): kT. Queue Pool (nc.gpsimd): x, weights.
        g8 = wp.tile([P, DC, D], fp8, tag="g8")
        nc.gpsimd.dma_start(g8[:].bitcast(mybir.dt.uint32), G[:, :])
        x_full = g_pool.tile([P, RC, D], f32, tag="x_full")
        wv8 = wp.tile([P, DC, D], fp8, tag="wv8")
        nc.sync.dma_start(wv8[:].bitcast(mybir.dt.uint32), Wv[:, :])
        w1_sb = wp.tile([P, DC, F], bf16, tag="w1")
        nc.sync.dma_start(w1_sb[:], W1[:, :])
        w2_sb = wp.tile([P, FC, D], bf16, tag="w2")
        nc.sync.dma_start(w2_sb[:], W2[:, :])
        b1_fm = wp.tile([P, FC], f32, tag="b1")
        nc.sync.dma_start(b1_fm[:], b1[:, :])
        if abias is not None:
            abias_fm = wp.tile([P, SC], f32, tag="abias")
            nc.sync.dma_start(abias_fm[:], abias[:, :])
        else:
            abias_fm = None

        with ExitStack() as actx:
            attn = actx.enter_context(tc.tile_pool(name="attn", bufs=1))
            kt8 = attn.tile([P, DC, S], fp8, tag="kt8")
            qgt = attn.tile([P, DC, M], fp8, tag="qgt")
            v8 = attn.tile([P, SC, D], fp8, tag="v8")

            # ---------- pass 1: direct fp8 loads + q-side G projection ----
            with ExitStack() as p1ctx:
                qt8_full = attn.tile([P, DC, M], fp8, tag="qt8f")
                u32 = mybir.dt.uint32
                nc.gpsimd.dma_start(qt8_full[:].bitcast(u32), qT[:, :])
                kr = kT[:, :].rearrange("p (c r) -> p c r", r=S // 4)
                k32 = kt8[:].bitcast(u32)
                nc.scalar.dma_start(k32[:, 0:2, :], kr[:, 0:2, :])
                nc.gpsimd.dma_start(k32[:, 2:4, :], kr[:, 2:4, :])
                vr = v[:, :].rearrange("p (c d) -> p c d", d=D // 4)
                v32 = v8[:].bitcast(u32)
                nc.gpsimd.dma_start(v32[:, 0:16, :], vr[:, 0:16, :])
                nc.gpsimd.dma_start(v32[:, 16:32, :], vr[:, 16:32, :])
                xr = x[:, :].rearrange("p (c d) -> p c d", d=D)
                nc.scalar.dma_start(x_full[:, 0:4, :], xr[:, 0:4, :])
                nc.sync.dma_start(x_full[:, 4:RC, :], xr[:, 4:RC, :])
                ps_p = p1ctx.enter_context(
                    tc.tile_pool(name="ps_p", bufs=3, space="PSUM"))
                for j in range(NQB):
                    for m in range(DC):
                        psp = ps_p.tile([P, QB], f32, tag="ps_proj")
                        for c2 in (0, 2):
                            nc.tensor.matmul(
                                psp[:],
                                lhsT=g8[:, c2:c2 + 2, m * P:(m + 1) * P],
                                rhs=qt8_full[:, c2:c2 + 2,
                                             j * QB:(j + 1) * QB],
                                start=(c2 == 0), stop=(c2 == 2),
                                perf_mode=DR)
                        nc.vector.tensor_copy(
                            qgt[:, m, j * QB:(j + 1) * QB], psp[:])

            # ---------- pass 2: attention; LN1 stats inline ----------
            with ExitStack() as p2ctx:
                p2 = p2ctx.enter_context(tc.tile_pool(name="ph2", bufs=1))
                ptp = p2ctx.enter_context(tc.tile_pool(name="ptp", bufs=3))
                zsp = p2ctx.enter_context(tc.tile_pool(name="zsp", bufs=2))
                ps_s = p2ctx.enter_context(
                    tc.tile_pool(name="ps_s", bufs=3, space="PSUM"))
                ps_z = p2ctx.enter_context(
                    tc.tile_pool(name="ps_z", bufs=1, space="PSUM"))
                ps_r = p2ctx.enter_context(
                    tc.tile_pool(name="ps_r", bufs=1, space="PSUM"))

                bv_bc = (None if bv is None else
                         _bcast_load(nc, p2, bv[:], D, "bv"))
                if "ln1_triv" not in spec:
                    g1a_bc = _bcast_load(nc, p2, g1[:], D, "g1a")
                    be1a_bc = _bcast_load(nc, p2, be1[:], D, "be1a")

                def _ln1_finish(lo, hi):
                    nc.scalar.activation(
                        out=mv1[:, lo:hi, 1:2], in_=mv1[:, lo:hi, 1:2],
                        func=Sqrt, bias=eps_t[:], scale=1.0, alpha=0.0)
                    nc.vector.reciprocal(out=mv1[:, lo:hi, 1],
                                         in_=mv1[:, lo:hi, 1])

                def _h_apply(rc):
                    hrow = h_bf[:, rc, :]
                    nc.vector.tensor_scalar(
                        out=hrow, in0=x_full[:, rc, :],
                        scalar1=mv1[:, rc, 0:1], scalar2=mv1[:, rc, 1:2],
                        op0=mybir.AluOpType.subtract,
                        op1=mybir.AluOpType.mult)
                    if "ln1_triv" not in spec:
                        nc.vector.tensor_mul(out=hrow, in0=hrow,
                                             in1=g1a_bc[:])
                        nc.vector.tensor_add(out=hrow, in0=hrow,
                                             in1=be1a_bc[:])

                for qb in range(NQB):
                    if qb == NQB - 1:
                        _ln1_finish(0, 12)
                    psZ = ps_z.tile([P, DC, QB], f32, tag="psZ")
                    pr = ps_r.tile([P, 4, 4], f32, tag="pr")
                    NP2 = SC // 2

                    def _zt_pair(kp, pt):
                        for dc in range(DC):
                            nc.tensor.matmul(
                                psZ[:, dc, :],
                                lhsT=v8[:, 2 * kp:2 * kp + 2,
                                        dc * P:(dc + 1) * P],
                                rhs=pt[:],
                                start=(kp == 0), stop=(kp == NP2 - 1),
                                perf_mode=DR)
                        for qc in range(4):
                            nc.tensor.matmul(
                                pr[:, qc, :],
                                lhsT=pt[:, :, qc * P:(qc + 1) * P],
                                rhs=ones8[:],
                                start=(kp == 0), stop=(kp == NP2 - 1),
                                perf_mode=DR)

                    pt_prev = None
                    for kp in range(NP2):
                        ptile = ptp.tile([P, 2, QB], fp8, tag="pt")
                        for sub in range(2):
                            kc = 2 * kp + sub
                            pss = ps_s.tile([P, QB], f32, tag="pss")
                            for d2 in (0, 2):
                                nc.tensor.matmul(
                                    pss[:],
                                    lhsT=kt8[:, d2:d2 + 2,
                                             kc * P:(kc + 1) * P],
                                    rhs=qgt[:, d2:d2 + 2,
                                            qb * QB:(qb + 1) * QB],
                                    start=(d2 == 0), stop=(d2 == 2),
                                    perf_mode=DR)
                            ebias = (negc_t[:] if abias_fm is None
                                     else abias_fm[:, kc:kc + 1])
                            nc.scalar.activation(
                                out=ptile[:, sub, :], in_=pss[:],
                                func=Exp, bias=ebias, scale=SCALE, alpha=0.0)
                        if pt_prev is not None:
                            _zt_pair(kp - 1, pt_prev)
                        pt_prev = ptile
                    _zt_pair(NP2 - 1, pt_prev)
                    rsum_sb = ep.tile([P, 4], f32, tag="rsum_sb")
                    nc.vector.tensor_copy(rsum_sb[:], pr[:, :, 0])
                    rinv = ep.tile([P, 4], f32, tag="rinv")
                    nc.vector.reciprocal(out=rinv[:], in_=rsum_sb[:])
                    z8 = zsp.tile([P, DC, QB], fp8, tag="z8")
                    psAs = []
                    for qc in range(4):
                        nc.vector.tensor_copy(
                            z8[:, :, qc * P:(qc + 1) * P],
                            psZ[:, :, qc * P:(qc + 1) * P])
                        psA = ps_s.tile([P, QB], f32, tag="pss", name="psA")
                        for c2 in (0, 2):
                            nc.tensor.matmul(
                                psA[:],
                                lhsT=z8[:, c2:c2 + 2, qc * P:(qc + 1) * P],
                                rhs=wv8[:, c2:c2 + 2, :],
                                start=(c2 == 0), stop=(c2 == 2),
                                perf_mode=DR)
                        psAs.append(psA)
                    if qb == NQB - 1:
                        for rc0 in range(4):
                            _h_apply(rc0)
                    for qc in range(4):
                        rc = qb * 4 + qc
                        t = x_full[:, rc, :]
                        nc.vector.scalar_tensor_tensor(
                            out=t, in0=psAs[qc][:],
                            scalar=rinv[:, qc:qc + 1],
                            in1=t, op0=mybir.AluOpType.mult,
                            op1=mybir.AluOpType.add)
                        if bv_bc is not None:
                            nc.vector.tensor_add(out=t, in0=t, in1=bv_bc[:])
                        stats = ep.tile([P, 6], f32, tag="ln_stats")
                        nc.vector.bn_stats(out=stats[:], in_=t)
                        nc.vector.bn_aggr(out=mv1[:, rc, :], in_=stats[:])

                _ln1_finish(12, RC)

        # ---------- pass 3: LN1-apply, FFN, LN2, store ----------
        with ExitStack() as p3ctx:
            p3 = p3ctx.enter_context(tc.tile_pool(name="ph3", bufs=1))
            f1p = p3ctx.enter_context(tc.tile_pool(name="f1p", bufs=1))
            ps_f = p3ctx.enter_context(
                tc.tile_pool(name="ps_f", bufs=2, space="PSUM"))
            ps_g = p3ctx.enter_context(
                tc.tile_pool(name="ps_g", bufs=3, space="PSUM"))
            ps_t = p3ctx.enter_context(
                tc.tile_pool(name="ps_t", bufs=2, space="PSUM"))

            if "ln1_triv" in spec:
                g1_bc = be1_bc = None
            else:
                g1_bc = _bcast_load(nc, p3, g1[:], D, "g1")
                be1_bc = _bcast_load(nc, p3, be1[:], D, "be1")
            if "ln2_triv" in spec:
                g2_bc = be2_bc = None
            else:
                g2_bc = _bcast_load(nc, p3, g2[:], D, "g2")
                be2_bc = _bcast_load(nc, p3, be2[:], D, "be2")
            b2_bc = (None if b2 is None else
                     _bcast_load(nc, p3, b2[:], D, "b2"))

            def _prep_block(fb):
                """LN1-apply each row into h_bf, PE-transpose it."""
                htr = htp.tile([P, DC, QB], bf16, tag="ht_blk",
                               name=f"htl{fb}")
                for qc in range(4):
                    rc = fb * 4 + qc
                    hrow = h_bf[:, rc, :]
                    if fb > 0:
                        nc.vector.tensor_scalar(
                            out=hrow, in0=x_full[:, rc, :],
                            scalar1=mv1[:, rc, 0:1], scalar2=mv1[:, rc, 1:2],
                            op0=mybir.AluOpType.subtract,
                            op1=mybir.AluOpType.mult)
                        if g1_bc is not None:
                            nc.vector.tensor_mul(out=hrow, in0=hrow,
                                                 in1=g1_bc[:])
                            nc.vector.tensor_add(out=hrow, in0=hrow,
                                                 in1=be1_bc[:])
                    pst = ps_t.tile([P, DC, P], bf16, tag="ps_tp")
                    for dc in range(DC):
                        nc.tensor.transpose(
                            pst[:, dc, :],
                            h_bf[:, rc, dc * P:(dc + 1) * P], ident)
                    nc.vector.tensor_copy(htr[:, :, qc * P:(qc + 1) * P],
                                          pst[:])
                return htr

            mv2 = p3.tile([P, RC, 2], f32, tag="mv2")
            ht_next = _prep_block(0)
            for fb in range(NQB):
                htr = ht_next
                f1t = f1p.tile([P, FC, QB], bf16, tag="f1t")
                for fc in range(FC):
                    psf = ps_f.tile([P, QB], f32, tag="ps_ffn")
                    for dc in range(DC):
                        nc.tensor.matmul(
                            psf[:], lhsT=w1_sb[:, dc, fc * P:(fc + 1) * P],
                            rhs=htr[:, dc, :],
                            start=(dc == 0), stop=(dc == DC - 1))
                    nc.scalar.activation(
                        out=f1t[:, fc, :], in_=psf[:],
                        func=Relu, bias=b1_fm[:, fc:fc + 1], scale=1.0,
                        alpha=0.0)
                if fb + 1 < NQB:
                    ht_next = _prep_block(fb + 1)
                u2 = p3.tile([P, 4, D], f32, tag="u2", name=f"u2_{fb}")
                last = fb == NQB - 1

                def _ln2_emit(qcs):
                    fs0 = fb * 4 + qcs[0]
                    fs1 = fb * 4 + qcs[-1] + 1
                    nc.scalar.activation(
                        out=mv2[:, fs0:fs1, 1:2], in_=mv2[:, fs0:fs1, 1:2],
                        func=Sqrt, bias=eps_t[:], scale=1.0, alpha=0.0)
                    nc.vector.reciprocal(out=mv2[:, fs0:fs1, 1],
                                         in_=mv2[:, fs0:fs1, 1])
                    for qc in qcs:
                        rc = fb * 4 + qc
                        res_t = ep.tile([P, D], f32, tag="res_t")
                        nc.vector.tensor_scalar(
                            out=res_t[:], in0=u2[:, qc, :],
                            scalar1=mv2[:, rc, 0:1], scalar2=mv2[:, rc, 1:2],
                            op0=mybir.AluOpType.subtract,
                            op1=mybir.AluOpType.mult)
                        if g2_bc is not None:
                            nc.vector.tensor_mul(out=res_t[:], in0=res_t[:],
                                                 in1=g2_bc[:])
                            nc.vector.tensor_add(out=res_t[:], in0=res_t[:],
                                                 in1=be2_bc[:])
                        eng = (nc.sync, nc.gpsimd, nc.scalar)[qc % 3]
                        eng.dma_start(out[rc * P:(rc + 1) * P, :], res_t[:])

                for qc in range(4):
                    rc = fb * 4 + qc
                    pso = ps_g.tile([P, D], f32, tag="ps_out2")
                    for fc in range(FC):
                        nc.tensor.matmul(
                            pso[:], lhsT=f1t[:, fc, qc * P:(qc + 1) * P],
                            rhs=w2_sb[:, fc, :],
                            start=(fc == 0), stop=(fc == FC - 1))
                    if b2_bc is not None:
                        nc.vector.tensor_add(out=pso[:], in0=pso[:],
                                             in1=b2_bc[:])
                    nc.vector.tensor_add(out=u2[:, qc, :], in0=pso[:],
                                         in1=h_bf[:, rc, :])
                    stats = ep.tile([P, 6], f32, tag="ln_stats")
                    nc.vector.bn_stats(out=stats[:], in_=u2[:, qc, :])
                    nc.vector.bn_aggr(out=mv2[:, rc, :], in_=stats[:])
                    if last and qc >= 2:
                        # drain rows eagerly at the end of the kernel
                        _ln2_emit(list(range(2)) if qc == 2 else [2, 3])
                if not last:
                    _ln2_emit([0, 1, 2, 3])

    nc.finalize()
    return nc


_CACHE = {}
_LOCK = threading.Lock()


def _get_program(spec):
    with _LOCK:
        if spec not in _CACHE:
            _CACHE[spec] = build_program(spec)
        return _CACHE[spec]


def _spec_flags(inputs):
    flags = set()
    if not np.any(inputs["bq"]):
        flags.add("bq0")
    if not np.any(inputs["bv"]):
        flags.add("bv0")
    if not np.any(inputs["b2"]):
        flags.add("b20")
    if (np.all(inputs["gamma1"] == 1.0) and not np.any(inputs["beta1"])):
        flags.add("ln1_triv")
    if (np.all(inputs["gamma2"] == 1.0) and not np.any(inputs["beta2"])):
        flags.add("ln2_triv")
    return frozenset(flags)


def _shuf(a, chunks):
    """[chunks*128, n] row-major -> [128, chunks*n] partition-major."""
    n = a.shape[1]
    return np.ascontiguousarray(
        a.reshape(chunks, P, n).transpose(1, 0, 2).reshape(P, chunks * n))


def make_in_maps(inputs):
    bf = ml_dtypes.bfloat16
    f8 = ml_dtypes.float8_e4m3
    spec = _spec_flags(inputs)
    # device projection applies G_dev^T to q^T, so upload Wq@Wk^T to get
    # scoresT = k (Wk Wq^T) q^T = K Q^T
    G = (inputs["Wq"].astype(np.float32)
         @ inputs["Wk"].astype(np.float32).T).astype(f8)
    weights = {
        "G": _shuf(G, DC).view(np.uint32),
        "Wv": _shuf(inputs["Wv"].astype(f8), DC).view(np.uint32),
        "W1": _shuf(inputs["W1"].astype(bf), DC),
        "W2": _shuf(inputs["W2"].astype(bf), FC),
        "b1": np.ascontiguousarray(
            inputs["b1"].astype(np.float32).reshape(FC, P).T),
    }
    if "bv0" not in spec:
        weights["bv"] = np.ascontiguousarray(inputs["bv"].astype(np.float32))
    if "b20" not in spec:
        weights["b2"] = np.ascontiguousarray(inputs["b2"].astype(np.float32))
    if "ln1_triv" not in spec:
        weights["gamma1"] = np.ascontiguousarray(
            inputs["gamma1"].astype(np.float32))
        weights["beta1"] = np.ascontiguousarray(
            inputs["beta1"].astype(np.float32))
    if "ln2_triv" not in spec:
        weights["gamma2"] = np.ascontiguousarray(
            inputs["gamma2"].astype(np.float32))
        weights["beta2"] = np.ascontiguousarray(
            inputs["beta2"].astype(np.float32))
    wkbq = (None if "bq0" in spec else
            inputs["Wk"].astype(np.float32)
            @ inputs["bq"].astype(np.float32))
    in_maps = []
    for c in range(N_CORES):
        b, h = c // 2, c % 2
        sl = slice(h * M, (h + 1) * M)
        kb = inputs["k"][b].astype(np.float32)
        m = {
            "qT": _shuf(np.ascontiguousarray(inputs["q"][b, sl].T)
                        .astype(bf).astype(f8), DC).view(np.uint32),
            "kT": _shuf(np.ascontiguousarray(kb.T)
                        .astype(bf).astype(f8), DC).view(np.uint32),
            "v": _shuf(inputs["v"][b].astype(bf).astype(f8),
                       SC).view(np.uint32),
            "x": _shuf(inputs["x"][b, sl].astype(np.float32), RC),
            **weights,
        }
        if wkbq is not None:
            m["abias"] = np.ascontiguousarray(
                ((kb @ wkbq) * SCALE - CSHIFT)
                .astype(np.float32).reshape(SC, P).T)
        in_maps.append(m)
    return in_maps


def kernel(**inputs):
    spec = _spec_flags(inputs)
    nc = _get_program(spec)
    in_maps = make_in_maps(inputs)
    res = run_bass_kernel_spmd(nc, in_maps, list(range(N_CORES)))
    out = np.empty((B, S, D), np.float32)
    for c in range(N_CORES):
        b, h = c // 2, c % 2
        out[b, h * M:(h + 1) * M] = res.results[c]["out"]
    return out
